# revision 18
# baseline (speedup 1.0000x reference)
"""Trainium2 Bass kernel for nn_Attention_Correlation_weight_reshape_loss.

loss = [ sum|real - C_real| + sum|fake - C_fake| ] / (B*(PP^2-PP))

Key identity: C_IN == C_OUT == 0.8, so with s[b,i] = +1 if fake_weight[b,i] > 0
else -1 the fake target is rank-1:
    C_fake[b,i,j] = 0.45 + 0.35 * s[b,i] * s[b,j]
and since s*s = +/-1:
    |fake - C_fake| = | (fake - 0.45)*s_i*s_j - 0.35 |
C_real = 0.8 everywhere except the diagonal (1.0); since x < 1, |x_d - 1| =
1 - x_d, so the diagonal correction needs only sum(diag) and sum|diag - 0.8|.

Per-core plan (data-parallel over batch, 8 cores x 128 batches), both maps in
the flat [batch=partition, 38416] layout -> every DMA descriptor is a >=2352 B
contiguous run. Two HWDGE rings stream in parallel (measured ~425 GB/s
aggregate): fake chunks on the Sync ring, real chunks on the Scalar ring, with
each ring's issue slots emitted so no data-waiting compute ever sits ahead of
a DMA issue on the same engine queue.

VectorE is the scarce engine (both fake passes are DVE work; 1 elem/cycle in
every dtype -- 16-bit gets no DVE speedup, and GpSimd elementwise throttles
the DVE via the shared SBUF ports, both measured). Balance:
  ScalarE:  real chunks Abs(x-0.8) activation w/ accumulate, and for a few
            fake chunks pass 1 done per-row: for one row s_i is a
            per-partition scalar, so Identity activation with
            scale=s[:,i], bias=-0.45*s[:,i] gives t = (x-0.45)*s_i.
  VectorE:  fake pass 1 via fused STT for the remaining chunks, fake pass 2
            |t * s - 0.35| custom DVE op w/ accumulate for all chunks
            (the sign not folded in pass 1 rides pass 2's broadcast),
            per-chunk diagonal gather + 2 tiny end reductions.
No compute op writes a DMA-landing tile in place (separate scratch outputs),
so buffer recycling never crosses engines on the critical path.
Host sums the [128, 36] partial tensor from each core and divides by denom.
"""

from operator import add as _op_add

import numpy as np

import concourse.bacc as bacc
import concourse.bass as bass
import concourse.mybir as mybir
import concourse.tile as tile
from concourse import bass_utils
from concourse import dve_ops as _dops
from concourse.dve_spec import Spec, Src0, Src1, Zero, maxx, lower
from concourse.dve_spec import _has_src1
from concourse import dve_spec as _dspec
from concourse.dve_uop import DveOpSpec


def _ensure_axon_ntff_shim():
    """Some agent images lack antenv.axon_hooks; run_bass_kernel_spmd
    (trace=True under axon) hard-imports it. Install a minimal shim wired
    to the axon .so so tracing works instead of crashing."""
    import sys
    import types

    try:
        import antenv.axon_hooks  # noqa: F401
        return
    except ImportError:
        pass
    try:
        import antenv
    except ImportError:
        return
    mod = types.ModuleType("antenv.axon_hooks")
    _hook = [None]
    mod.set_axon_ntff_profile_hook = lambda h: _hook.__setitem__(0, h)
    mod.get_axon_ntff_profile_hook = lambda: _hook[0]
    sys.modules["antenv.axon_hooks"] = mod
    antenv.axon_hooks = mod
    try:
        from trn_agent_boot.trn_boot import _ntff_profile_via_ctypes

        mod.set_axon_ntff_profile_hook(
            _ntff_profile_via_ctypes("/opt/axon/libaxon_pjrt.so")
        )
    except Exception:
        pass


_ensure_axon_ntff_shim()

F32 = mybir.dt.float32
AF = mybir.ActivationFunctionType
ALU = mybir.AluOpType

B, PP = 1024, 196
NCORES = 8
BS = B // NCORES            # 128 batches per core
FF = PP * PP                # 38416
RF = 2744                   # full chunk: 14 rows of 196

# chunk schedule: two 7-row lead-in chunks (compute starts sooner), 12 full
# 14-row chunks, then 7/4/3-row tail pieces (fine drain). All boundaries are
# multiples of 196 so the fake-map row structure stays intact.
CHUNKS = (
    [(0, 7 * PP), (7 * PP, 7 * PP)]
    + [(RF + c * RF, RF) for c in range(12)]
    + [(13 * RF, 7 * PP), (13 * RF + 7 * PP, 4 * PP), (13 * RF + 11 * PP, 3 * PP)]
)
NCH = len(CHUNKS)           # 17

# fake chunks whose pass 1 runs per-row on ScalarE (full chunks, spread out)
SC_P1 = frozenset({3, 6, 9, 12})

# output partials layout: [128, NCOL]
COL_REAL = 0                # NCH cols: per-chunk sum|x-0.8| (incl diag)
COL_FAKE = NCH              # NCH cols: per-chunk fake sums
COL_SD = 2 * NCH            # sum(diag)        (diag target 1.0: |d-1| = 1-d)
COL_D8 = COL_SD + 1         # sum|diag-0.8|
NCOL = COL_D8 + 1           # 36

DENOM = float(B) * (FF - PP)

_NC_CACHE = {}


def _register_op(name, body_fn, ref_fn):
    for op in _dops.OPS:
        if op.name == name:
            return op
    spec = Spec(body=body_fn(), accum=_op_add, accum_init=Zero, reference=ref_fn)
    row = max(_dops._SUB_OPCODE_FOR_NAME.values()) + 1
    assert row < 0x20
    _dops._SUB_OPCODE_FOR_NAME[name] = row
    shas = {}
    for ver in ("v3", "v4"):
        s = DveOpSpec(
            name=name, opcode=row, uops=lower(spec, ver=ver),
            rd1_en=_has_src1(spec),
        )
        shas[ver] = s.sha(ver)
    op = _dops.DveOp(name, spec, subdim=False, uops_sha=shas)
    _dops.OPS.append(op)
    _dops.CUSTOM_DVE_SPECS[name] = spec
    return op


def _register_mul_absdiff_op():
    """out = |in0*in1 - s0|, accum_out = row-sum(out)."""

    def _body():
        e = (Src0 * Src1) - _dspec.C0
        return maxx(e, Zero - e)

    def _ref(in0, in1, c0, c1, c2):
        P = in0.shape[0]
        a = np.asarray(in0, dtype=np.float32).reshape(P, -1)
        x = np.asarray(in1, dtype=np.float32).reshape(P, -1)
        bb = np.abs(a * x - c0).astype(np.float32)
        return bb, bb.sum(axis=-1, keepdims=True)

    return _register_op("MUL_ABSDIFF_SUM_ANT", _body, _ref)


def build_nc():
    mad_op = _register_mul_absdiff_op()
    nc = bacc.Bacc(
        "TRN2", target_bir_lowering=False, debug=False, enable_asserts=False
    )
    real = nc.dram_tensor("real", [BS, FF], F32, kind="ExternalInput").ap()
    fake = nc.dram_tensor("fake", [BS, FF], F32, kind="ExternalInput").ap()
    fw = nc.dram_tensor("fw", [BS, PP], F32, kind="ExternalInput").ap()
    out = nc.dram_tensor("out", [128, NCOL], F32, kind="ExternalOutput").ap()

    with tile.TileContext(nc) as tc:
        with (
            tc.tile_pool(name="small", bufs=1) as sp,
            tc.tile_pool(name="xr", bufs=6) as xr_pool,
            tc.tile_pool(name="xfc", bufs=7) as xfc_pool,
            tc.tile_pool(name="t", bufs=2) as t_pool,
        ):
            O = sp.tile([128, NCOL], F32)

            # --- fw via the Sync HWDGE ring, ahead of the fake chunks
            fwt = sp.tile([128, PP], F32)
            nc.sync.dma_start(fwt[:], fw[:, :])

            # all fake-chunk loads up front on the Sync ring; the WAR waits
            # (xfc buffer recycle) throttle them in place
            xfc_tiles = []
            for c, (lo, sz) in enumerate(CHUNKS):
                xfc = xfc_pool.tile([128, RF], F32, tag="xfc")
                nc.sync.dma_start(xfc[:, 0:sz], fake[:, lo : lo + sz])
                xfc_tiles.append(xfc)

            # first few real-chunk loads on the Scalar ring
            xr_tiles = [None] * NCH
            PRE_R = 4

            def issue_real(c):
                lo, sz = CHUNKS[c]
                xr = xr_pool.tile([128, RF], F32, tag="xr")
                nc.scalar.dma_start(xr[:, 0:sz], real[:, lo : lo + sz])
                xr_tiles[c] = xr

            for c in range(PRE_R):
                issue_real(c)

            # s prep: s = +/-1 from fw > 0; b45 = -0.45*s (per-row act bias)
            g_t = sp.tile([128, PP], F32)
            nc.vector.tensor_scalar(g_t[:], fwt[:], 0.0, None, ALU.is_gt)
            s_t = sp.tile([128, PP], F32)
            nc.vector.tensor_scalar(s_t[:], g_t[:], 2.0, 1.0, ALU.mult, ALU.subtract)
            b45 = sp.tile([128, PP], F32)
            nc.vector.tensor_scalar(b45[:], s_t[:], -0.45, None, ALU.mult)

            diag = sp.tile([128, PP], F32)
            sa = sp.tile([128, RF], F32)    # ScalarE activation out scratch
            sd = sp.tile([128, RF], F32)    # VectorE custom-op out scratch
            b08 = sp.tile([128, 1], F32)    # activation bias (-0.8)
            nc.vector.memset(b08[:], -0.8)
            ones = sp.tile([128, 1], F32)   # broadcast 1.0 for the D8 reduce
            nc.vector.memset(ones[:], 1.0)

            def fake_pass1(c):
                lo, sz = CHUNKS[c]
                rows = sz // PP
                r0 = lo // PP
                t = t_pool.tile([128, RF], F32, tag="t")
                if c in SC_P1:
                    # per-row on ScalarE: t_i = s_i*x_i - 0.45*s_i
                    for i in range(rows):
                        nc.scalar.activation(
                            t[:, i * PP : (i + 1) * PP],
                            xfc_tiles[c][:, i * PP : (i + 1) * PP],
                            AF.Identity,
                            bias=b45[:, r0 + i : r0 + i + 1],
                            scale=s_t[:, r0 + i : r0 + i + 1],
                        )
                else:
                    x3 = xfc_tiles[c][:, 0:sz].rearrange(
                        "p (i j) -> p i j", j=PP
                    )
                    t3 = t[:, 0:sz].rearrange("p (i j) -> p i j", j=PP)
                    sjb = s_t[:].rearrange("p j -> p () j").to_broadcast(
                        [128, rows, PP]
                    )
                    nc.vector.scalar_tensor_tensor(
                        t3, x3, 0.45, sjb, ALU.subtract, ALU.mult
                    )
                return t

            def fake_pass2(c, t):
                lo, sz = CHUNKS[c]
                rows = sz // PP
                r0 = lo // PP
                t3 = t[:, 0:sz].rearrange("p (i j) -> p i j", j=PP)
                if c in SC_P1:
                    # pass 1 folded s_i; fold s_j here
                    sb = s_t[:].rearrange("p j -> p () j").to_broadcast(
                        [128, rows, PP]
                    )
                else:
                    sb = (
                        s_t[:, r0 : r0 + rows]
                        .rearrange("p i -> p i ()")
                        .to_broadcast([128, rows, PP])
                    )
                nc.vector._custom_dve(
                    mad_op,
                    out=sd[:, 0:sz].rearrange("p (i j) -> p i j", j=PP),
                    in0=t3,
                    in1=sb,
                    s0=0.35,
                    accum_out=O[:, COL_FAKE + c : COL_FAKE + c + 1],
                )

            def diag_copy(c):
                lo, sz = CHUNKS[c]
                i0 = -(-lo // 197)
                i1 = -(-(lo + sz) // 197)
                off = 197 * i0 - lo
                cnt = i1 - i0
                nc.vector.tensor_copy(
                    diag[:, i0:i1],
                    xr_tiles[c][:, off : off + 197 * (cnt - 1) + 1 : 197],
                )

            def real_act(c):
                lo, sz = CHUNKS[c]
                nc.scalar.activation(
                    sa[:, 0:sz], xr_tiles[c][:, 0:sz], AF.Abs, bias=b08[:],
                    accum_out=O[:, COL_REAL + c : COL_REAL + c + 1],
                )

            t_tiles = [None] * NCH
            for c in range(NCH):
                t_tiles[c] = fake_pass1(c)
                real_act(c)
                if c + PRE_R < NCH:
                    issue_real(c + PRE_R)
                if c >= 1:
                    diag_copy(c - 1)
                    fake_pass2(c - 1, t_tiles[c - 1])
                    t_tiles[c - 1] = None
            diag_copy(NCH - 1)
            fake_pass2(NCH - 1, t_tiles[NCH - 1])

            # diagonal corrections on VectorE (real diag target is 1.0):
            # ship sum(diag) and sum|diag - 0.8|; host applies 196 - sum(d).
            nc.vector.tensor_scalar(
                g_t[:], diag[:], 0.0, 0.0, ALU.add, ALU.add,
                accum_out=O[:, COL_SD : COL_SD + 1],
            )
            nc.vector._custom_dve(
                mad_op,
                out=fwt[:],
                in0=diag[:],
                in1=ones[:].to_broadcast([128, PP]),
                s0=0.8,
                accum_out=O[:, COL_D8 : COL_D8 + 1],
            )

            nc.sync.dma_start(out[:, :], O[:])

    nc.compile()
    return nc


def _get_nc():
    if "nc" not in _NC_CACHE:
        _NC_CACHE["nc"] = build_nc()
    return _NC_CACHE["nc"]


def make_in_maps(correlation_map_real, correlation_map_fake, fake_weight):
    r = np.ascontiguousarray(correlation_map_real, dtype=np.float32).reshape(B, FF)
    f = np.ascontiguousarray(correlation_map_fake, dtype=np.float32).reshape(B, FF)
    w = np.ascontiguousarray(fake_weight, dtype=np.float32).reshape(B, PP)
    return [
        {
            "real": r[k * BS : (k + 1) * BS],
            "fake": f[k * BS : (k + 1) * BS],
            "fw": w[k * BS : (k + 1) * BS],
        }
        for k in range(NCORES)
    ]


def reduce_outputs(results):
    total = 0.0
    for k in range(NCORES):
        Ov = results[k]["out"].astype(np.float64)
        total += (
            Ov[:, COL_REAL : COL_REAL + NCH].sum()
            + Ov[:, COL_FAKE : COL_FAKE + NCH].sum()
            + (BS * PP - Ov[:, COL_SD].sum())   # sum(1 - d) over the diag
            - Ov[:, COL_D8].sum()
        )
    return np.float32(total / DENOM)


def run(inputs, trace=False, **kwargs):
    nc = _get_nc()
    in_maps = make_in_maps(**inputs)
    res = bass_utils.run_bass_kernel_spmd(
        nc, in_maps, list(range(NCORES)), trace=trace, **kwargs
    )
    return reduce_outputs(res.results), res


def kernel(correlation_map_real, correlation_map_fake, fake_weight):
    loss, _ = run(
        dict(
            correlation_map_real=correlation_map_real,
            correlation_map_fake=correlation_map_fake,
            fake_weight=fake_weight,
        )
    )
    return loss


# revision 27
# speedup vs baseline: 1.1296x; 1.1296x over previous
"""Trainium2 Bass kernel for nn_Attention_Correlation_weight_reshape_loss.

loss = [ sum|real - C_real| + sum|fake - C_fake| ] / (B*(PP^2-PP))

Key identity: C_IN == C_OUT == 0.8, so with s[b,i] = +1 if fake_weight[b,i] > 0
else -1 the fake target is rank-1:
    C_fake[b,i,j] = 0.45 + 0.35 * s[b,i] * s[b,j]
and since s*s = +/-1:
    |fake - C_fake| = | (fake - 0.45)*s_i*s_j - 0.35 |
C_real = 0.8 everywhere except the diagonal (1.0); since x < 1, |x_d - 1| =
1 - x_d, so the diagonal correction needs only sum(diag) and sum|diag - 0.8|.

Per-core plan (data-parallel over batch, 8 cores x 128 batches), both maps in
the flat [batch=partition, 38416] layout -> every DMA descriptor is a >=2352 B
contiguous run. Two HWDGE rings stream in parallel (measured ~425 GB/s
aggregate): fake chunks on the Sync ring, real chunks on the Scalar ring, with
each ring's issue slots emitted so no data-waiting compute ever sits ahead of
a DMA issue on the same engine queue.

VectorE is the scarce engine (both fake passes are DVE work; 1 elem/cycle in
every dtype -- 16-bit gets no DVE speedup, and GpSimd elementwise throttles
the DVE via the shared SBUF ports, both measured). Balance:
  ScalarE:  real chunks Abs(x-0.8) activation w/ accumulate, and for a few
            fake chunks pass 1 done per-row: for one row s_i is a
            per-partition scalar, so Identity activation with
            scale=s[:,i], bias=-0.45*s[:,i] gives t = (x-0.45)*s_i.
  VectorE:  fake pass 1 via fused STT for the remaining chunks, fake pass 2
            |t * s - 0.35| custom DVE op w/ accumulate for all chunks
            (the sign not folded in pass 1 rides pass 2's broadcast),
            per-chunk diagonal gather + 2 tiny end reductions.
No compute op writes a DMA-landing tile in place (separate scratch outputs),
so buffer recycling never crosses engines on the critical path.
Host sums the [128, 36] partial tensor from each core and divides by denom.
"""

from operator import add as _op_add

import numpy as np

import concourse.bacc as bacc
import concourse.bass as bass
import concourse.mybir as mybir
import concourse.tile as tile
from concourse import bass_utils
from concourse import dve_ops as _dops
from concourse.dve_spec import Spec, Src0, Src1, Zero, maxx, lower
from concourse.dve_spec import _has_src1
from concourse import dve_spec as _dspec
from concourse.dve_uop import DveOpSpec


def _ensure_axon_ntff_shim():
    """Some agent images lack antenv.axon_hooks; run_bass_kernel_spmd
    (trace=True under axon) hard-imports it. Install a minimal shim wired
    to the axon .so so tracing works instead of crashing."""
    import sys
    import types

    try:
        import antenv.axon_hooks  # noqa: F401
        return
    except ImportError:
        pass
    try:
        import antenv
    except ImportError:
        return
    mod = types.ModuleType("antenv.axon_hooks")
    _hook = [None]
    mod.set_axon_ntff_profile_hook = lambda h: _hook.__setitem__(0, h)
    mod.get_axon_ntff_profile_hook = lambda: _hook[0]
    sys.modules["antenv.axon_hooks"] = mod
    antenv.axon_hooks = mod
    try:
        from trn_agent_boot.trn_boot import _ntff_profile_via_ctypes

        mod.set_axon_ntff_profile_hook(
            _ntff_profile_via_ctypes("/opt/axon/libaxon_pjrt.so")
        )
    except Exception:
        pass


_ensure_axon_ntff_shim()

F32 = mybir.dt.float32
AF = mybir.ActivationFunctionType
ALU = mybir.AluOpType

B, PP = 1024, 196
NCORES = 8
BS = B // NCORES            # 128 batches per core
FF = PP * PP                # 38416
RF = 2744                   # full chunk: 14 rows of 196

# chunk schedule: two 7-row lead-in chunks (compute starts sooner), 12 full
# 14-row chunks, then 7/4/3-row tail pieces (fine drain). All boundaries are
# multiples of 196 so the fake-map row structure stays intact.
CHUNKS = (
    [(0, 7 * PP), (7 * PP, 7 * PP)]
    + [(RF + c * RF, RF) for c in range(12)]
    + [(13 * RF, 7 * PP), (13 * RF + 7 * PP, 4 * PP), (13 * RF + 11 * PP, 3 * PP)]
)
NCH = len(CHUNKS)           # 17

# fake chunks whose pass 1 runs per-row on ScalarE (full chunks, spread out)
SC_P1 = frozenset({3, 9})

# output partials layout: [128, NCOL]
COL_REAL = 0                # NCH cols: per-chunk sum|x-0.8| (incl diag)
COL_FAKE = NCH              # NCH cols: per-chunk fake sums
COL_SD = 2 * NCH            # sum(diag)        (diag target 1.0: |d-1| = 1-d)
COL_D8 = COL_SD + 1         # sum|diag-0.8|
NCOL = COL_D8 + 1           # 36

DENOM = float(B) * (FF - PP)

_NC_CACHE = {}


def _register_op(name, body_fn, ref_fn):
    for op in _dops.OPS:
        if op.name == name:
            return op
    spec = Spec(body=body_fn(), accum=_op_add, accum_init=Zero, reference=ref_fn)
    row = max(_dops._SUB_OPCODE_FOR_NAME.values()) + 1
    assert row < 0x20
    _dops._SUB_OPCODE_FOR_NAME[name] = row
    shas = {}
    for ver in ("v3", "v4"):
        s = DveOpSpec(
            name=name, opcode=row, uops=lower(spec, ver=ver),
            rd1_en=_has_src1(spec),
        )
        shas[ver] = s.sha(ver)
    op = _dops.DveOp(name, spec, subdim=False, uops_sha=shas)
    _dops.OPS.append(op)
    _dops.CUSTOM_DVE_SPECS[name] = spec
    return op


def _register_mul_absdiff_op():
    """out = |in0*in1 - s0|, accum_out = row-sum(out)."""

    def _body():
        e = (Src0 * Src1) - _dspec.C0
        return maxx(e, Zero - e)

    def _ref(in0, in1, c0, c1, c2):
        P = in0.shape[0]
        a = np.asarray(in0, dtype=np.float32).reshape(P, -1)
        x = np.asarray(in1, dtype=np.float32).reshape(P, -1)
        bb = np.abs(a * x - c0).astype(np.float32)
        return bb, bb.sum(axis=-1, keepdims=True)

    return _register_op("MUL_ABSDIFF_SUM_ANT", _body, _ref)


def build_nc():
    mad_op = _register_mul_absdiff_op()
    nc = bacc.Bacc(
        "TRN2", target_bir_lowering=False, debug=False, enable_asserts=False
    )
    real = nc.dram_tensor("real", [BS, FF], F32, kind="ExternalInput").ap()
    fake = nc.dram_tensor("fake", [BS, FF], F32, kind="ExternalInput").ap()
    fw = nc.dram_tensor("fw", [BS, PP], F32, kind="ExternalInput").ap()
    out = nc.dram_tensor("out", [128, NCOL], F32, kind="ExternalOutput").ap()

    with tile.TileContext(nc) as tc:
        with (
            tc.tile_pool(name="small", bufs=1) as sp,
            tc.tile_pool(name="xr", bufs=6) as xr_pool,
            tc.tile_pool(name="xfc", bufs=8) as xfc_pool,
            tc.tile_pool(name="t", bufs=2) as t_pool,
        ):
            O = sp.tile([128, NCOL], F32)

            # --- fw via the Sync HWDGE ring, ahead of the fake chunks
            fwt = sp.tile([128, PP], F32)

            # fake-chunk loads on the Sync ring: prefetch a few, then issue
            # the rest one per loop iteration (just-in-time, baseline-style)
            xfc_tiles = [None] * NCH
            PRE_F = 6

            def issue_fake(c):
                lo, sz = CHUNKS[c]
                xfc = xfc_pool.tile([128, RF], F32, tag="xfc")
                nc.sync.dma_start(xfc[:, 0:sz], fake[:, lo : lo + sz])
                xfc_tiles[c] = xfc

            for c in range(PRE_F):
                issue_fake(c)

            # first few real-chunk loads on the Scalar ring
            xr_tiles = [None] * NCH
            PRE_R = 4

            def issue_real(c):
                lo, sz = CHUNKS[c]
                xr = xr_pool.tile([128, RF], F32, tag="xr")
                nc.scalar.dma_start(xr[:, 0:sz], real[:, lo : lo + sz])
                xr_tiles[c] = xr

            for c in range(PRE_R):
                issue_real(c)

            # s prep: s = +/-1 from fw > 0; b45 = -0.45*s (per-row act bias)
            g_t = sp.tile([128, PP], F32)
            nc.vector.tensor_scalar(g_t[:], fwt[:], 0.0, None, ALU.is_gt)
            s_t = sp.tile([128, PP], F32)
            nc.vector.tensor_scalar(s_t[:], g_t[:], 2.0, 1.0, ALU.mult, ALU.subtract)
            b45 = sp.tile([128, PP], F32)
            nc.vector.tensor_scalar(b45[:], s_t[:], -0.45, None, ALU.mult)
            b45 = sp.tile([128, PP], F32)
            nc.vector.tensor_scalar(b45[:], s_t[:], -0.45, None, ALU.mult)

            diag = sp.tile([128, PP], F32)
            sa = sp.tile([128, RF], F32)    # ScalarE activation out scratch
            sd = sp.tile([128, RF], F32)    # VectorE custom-op out scratch
            b08 = sp.tile([128, 1], F32)    # activation bias (-0.8)
            nc.vector.memset(b08[:], -0.8)
            ones = sp.tile([128, 1], F32)   # broadcast 1.0 for the D8 reduce
            nc.vector.memset(ones[:], 1.0)

            def fake_pass1(c):
                lo, sz = CHUNKS[c]
                rows = sz // PP
                r0 = lo // PP
                t = t_pool.tile([128, RF], F32, tag="t")
                if c in SC_P1:
                    # per-row on ScalarE: t_i = s_i*x_i - 0.45*s_i
                    for i in range(rows):
                        nc.scalar.activation(
                            t[:, i * PP : (i + 1) * PP],
                            xfc_tiles[c][:, i * PP : (i + 1) * PP],
                            AF.Identity,
                            bias=b45[:, r0 + i : r0 + i + 1],
                            scale=s_t[:, r0 + i : r0 + i + 1],
                        )
                else:
                    x3 = xfc_tiles[c][:, 0:sz].rearrange(
                        "p (i j) -> p i j", j=PP
                    )
                    t3 = t[:, 0:sz].rearrange("p (i j) -> p i j", j=PP)
                    sjb = s_t[:].rearrange("p j -> p () j").to_broadcast(
                        [128, rows, PP]
                    )
                    nc.vector.scalar_tensor_tensor(
                        t3, x3, 0.45, sjb, ALU.subtract, ALU.mult
                    )
                return t

            def fake_pass2(c, t):
                lo, sz = CHUNKS[c]
                rows = sz // PP
                r0 = lo // PP
                t3 = t[:, 0:sz].rearrange("p (i j) -> p i j", j=PP)
                if c in SC_P1:
                    # pass 1 folded s_i; fold s_j here
                    sb = s_t[:].rearrange("p j -> p () j").to_broadcast(
                        [128, rows, PP]
                    )
                else:
                    sb = (
                        s_t[:, r0 : r0 + rows]
                        .rearrange("p i -> p i ()")
                        .to_broadcast([128, rows, PP])
                    )
                nc.vector._custom_dve(
                    mad_op,
                    out=sd[:, 0:sz].rearrange("p (i j) -> p i j", j=PP),
                    in0=t3,
                    in1=sb,
                    s0=0.35,
                    accum_out=O[:, COL_FAKE + c : COL_FAKE + c + 1],
                )

            def diag_copy(c):
                lo, sz = CHUNKS[c]
                i0 = -(-lo // 197)
                i1 = -(-(lo + sz) // 197)
                off = 197 * i0 - lo
                cnt = i1 - i0
                nc.vector.tensor_copy(
                    diag[:, i0:i1],
                    xr_tiles[c][:, off : off + 197 * (cnt - 1) + 1 : 197],
                )

            def real_act(c):
                lo, sz = CHUNKS[c]
                nc.scalar.activation(
                    sa[:, 0:sz], xr_tiles[c][:, 0:sz], AF.Abs, bias=b08[:],
                    accum_out=O[:, COL_REAL + c : COL_REAL + c + 1],
                )

            t_tiles = [None] * NCH
            for c in range(NCH):
                t_tiles[c] = fake_pass1(c)
                real_act(c)
                if c + PRE_R < NCH:
                    issue_real(c + PRE_R)
                if c + PRE_F < NCH:
                    issue_fake(c + PRE_F)
                if c >= 1:
                    diag_copy(c - 1)
                    fake_pass2(c - 1, t_tiles[c - 1])
                    t_tiles[c - 1] = None
            diag_copy(NCH - 1)
            fake_pass2(NCH - 1, t_tiles[NCH - 1])

            # diagonal corrections on ScalarE (real diag target is 1.0):
            # ship sum(diag) and sum|diag - 0.8|; host applies 196 - sum(d).
            t1 = sp.tile([128, PP], F32)
            nc.scalar.activation(
                t1[:], diag[:], AF.Abs, bias=b00[:],
                accum_out=O[:, COL_SD : COL_SD + 1],
            )
            t2 = sp.tile([128, PP], F32)
            nc.scalar.activation(
                t2[:], diag[:], AF.Abs, bias=b08[:],
                accum_out=O[:, COL_D8 : COL_D8 + 1],
            )

            nc.sync.dma_start(out[:, :], O[:])

    nc.compile()
    return nc


def _get_nc():
    if "nc" not in _NC_CACHE:
        _NC_CACHE["nc"] = build_nc()
    return _NC_CACHE["nc"]


def make_in_maps(correlation_map_real, correlation_map_fake, fake_weight):
    r = np.ascontiguousarray(correlation_map_real, dtype=np.float32).reshape(B, FF)
    f = np.ascontiguousarray(correlation_map_fake, dtype=np.float32).reshape(B, FF)
    w = np.ascontiguousarray(fake_weight, dtype=np.float32).reshape(B, PP)
    return [
        {
            "real": r[k * BS : (k + 1) * BS],
            "fake": f[k * BS : (k + 1) * BS],
            "fw": w[k * BS : (k + 1) * BS],
        }
        for k in range(NCORES)
    ]


def reduce_outputs(results):
    total = 0.0
    for k in range(NCORES):
        Ov = results[k]["out"].astype(np.float64)
        total += (
            Ov[:, COL_REAL : COL_REAL + NCH].sum()
            + Ov[:, COL_FAKE : COL_FAKE + NFCH].sum()
            + (BS * PP - Ov[:, COL_SD].sum())   # sum(1 - d) over the diag
            - Ov[:, COL_D8].sum()
        )
    return np.float32(total / DENOM)


def run(inputs, trace=False, **kwargs):
    nc = _get_nc()
    in_maps = make_in_maps(**inputs)
    res = bass_utils.run_bass_kernel_spmd(
        nc, in_maps, list(range(NCORES)), trace=trace, **kwargs
    )
    return reduce_outputs(res.results), res


def kernel(correlation_map_real, correlation_map_fake, fake_weight):
    loss, _ = run(
        dict(
            correlation_map_real=correlation_map_real,
            correlation_map_fake=correlation_map_fake,
            fake_weight=fake_weight,
        )
    )
    return loss


# revision 28
# speedup vs baseline: 1.1297x; 1.0001x over previous
"""Trainium2 Bass kernel for nn_Attention_Correlation_weight_reshape_loss.

loss = [ sum|real - C_real| + sum|fake - C_fake| ] / (B*(PP^2-PP))

Key identity: C_IN == C_OUT == 0.8, so with s[b,i] = +1 if fake_weight[b,i] > 0
else -1 the fake target is rank-1:
    C_fake[b,i,j] = 0.45 + 0.35 * s[b,i] * s[b,j]
and since s*s = +/-1:
    |fake - C_fake| = | (fake - 0.45)*s_i*s_j - 0.35 |
C_real = 0.8 everywhere except the diagonal (1.0); since x < 1, |x_d - 1| =
1 - x_d, so the diagonal correction needs only sum(diag) and sum|diag - 0.8|.

Per-core plan (data-parallel over batch, 8 cores x 128 batches), both maps in
the flat [batch=partition, 38416] layout -> every DMA descriptor is a >=2352 B
contiguous run. Two HWDGE rings stream in parallel (measured ~425 GB/s
aggregate): fake chunks on the Sync ring, real chunks on the Scalar ring, with
each ring's issue slots emitted so no data-waiting compute ever sits ahead of
a DMA issue on the same engine queue.

VectorE is the scarce engine (both fake passes are DVE work; 1 elem/cycle in
every dtype -- 16-bit gets no DVE speedup, and GpSimd elementwise throttles
the DVE via the shared SBUF ports, both measured). Balance:
  ScalarE:  real chunks Abs(x-0.8) activation w/ accumulate, and for a few
            fake chunks pass 1 done per-row: for one row s_i is a
            per-partition scalar, so Identity activation with
            scale=s[:,i], bias=-0.45*s[:,i] gives t = (x-0.45)*s_i.
  VectorE:  fake pass 1 via fused STT for the remaining chunks, fake pass 2
            |t * s - 0.35| custom DVE op w/ accumulate for all chunks
            (the sign not folded in pass 1 rides pass 2's broadcast),
            per-chunk diagonal gather + 2 tiny end reductions.
No compute op writes a DMA-landing tile in place (separate scratch outputs),
so buffer recycling never crosses engines on the critical path.
Host sums the [128, 36] partial tensor from each core and divides by denom.
"""

from operator import add as _op_add

import numpy as np

import concourse.bacc as bacc
import concourse.bass as bass
import concourse.mybir as mybir
import concourse.tile as tile
from concourse import bass_utils
from concourse import dve_ops as _dops
from concourse.dve_spec import Spec, Src0, Src1, Zero, maxx, lower
from concourse.dve_spec import _has_src1
from concourse import dve_spec as _dspec
from concourse.dve_uop import DveOpSpec


def _ensure_axon_ntff_shim():
    """Some agent images lack antenv.axon_hooks; run_bass_kernel_spmd
    (trace=True under axon) hard-imports it. Install a minimal shim wired
    to the axon .so so tracing works instead of crashing."""
    import sys
    import types

    try:
        import antenv.axon_hooks  # noqa: F401
        return
    except ImportError:
        pass
    try:
        import antenv
    except ImportError:
        return
    mod = types.ModuleType("antenv.axon_hooks")
    _hook = [None]
    mod.set_axon_ntff_profile_hook = lambda h: _hook.__setitem__(0, h)
    mod.get_axon_ntff_profile_hook = lambda: _hook[0]
    sys.modules["antenv.axon_hooks"] = mod
    antenv.axon_hooks = mod
    try:
        from trn_agent_boot.trn_boot import _ntff_profile_via_ctypes

        mod.set_axon_ntff_profile_hook(
            _ntff_profile_via_ctypes("/opt/axon/libaxon_pjrt.so")
        )
    except Exception:
        pass


_ensure_axon_ntff_shim()

F32 = mybir.dt.float32
AF = mybir.ActivationFunctionType
ALU = mybir.AluOpType

B, PP = 1024, 196
NCORES = 8
BS = B // NCORES            # 128 batches per core
FF = PP * PP                # 38416
RF = 2744                   # full chunk: 14 rows of 196

# chunk schedule: two 7-row lead-in chunks (compute starts sooner), 12 full
# 14-row chunks, then 7/4/3-row tail pieces (fine drain). All boundaries are
# multiples of 196 so the fake-map row structure stays intact.
CHUNKS = (
    [(0, 7 * PP), (7 * PP, 7 * PP)]
    + [(RF + c * RF, RF) for c in range(12)]
    + [(13 * RF, 7 * PP), (13 * RF + 7 * PP, 4 * PP), (13 * RF + 11 * PP, 3 * PP)]
)
NCH = len(CHUNKS)           # 17

# fake chunks whose pass 1 runs per-row on ScalarE (full chunks, spread out)
SC_P1 = frozenset({3, 9})

# output partials layout: [128, NCOL]
COL_REAL = 0                # NCH cols: per-chunk sum|x-0.8| (incl diag)
COL_FAKE = NCH              # NCH cols: per-chunk fake sums
COL_SD = 2 * NCH            # sum(diag)        (diag target 1.0: |d-1| = 1-d)
COL_D8 = COL_SD + 1         # sum|diag-0.8|
NCOL = COL_D8 + 1           # 36

DENOM = float(B) * (FF - PP)

_NC_CACHE = {}


def _register_op(name, body_fn, ref_fn):
    for op in _dops.OPS:
        if op.name == name:
            return op
    spec = Spec(body=body_fn(), accum=_op_add, accum_init=Zero, reference=ref_fn)
    row = max(_dops._SUB_OPCODE_FOR_NAME.values()) + 1
    assert row < 0x20
    _dops._SUB_OPCODE_FOR_NAME[name] = row
    shas = {}
    for ver in ("v3", "v4"):
        s = DveOpSpec(
            name=name, opcode=row, uops=lower(spec, ver=ver),
            rd1_en=_has_src1(spec),
        )
        shas[ver] = s.sha(ver)
    op = _dops.DveOp(name, spec, subdim=False, uops_sha=shas)
    _dops.OPS.append(op)
    _dops.CUSTOM_DVE_SPECS[name] = spec
    return op


def _register_mul_absdiff_op():
    """out = |in0*in1 - s0|, accum_out = row-sum(out)."""

    def _body():
        e = (Src0 * Src1) - _dspec.C0
        return maxx(e, Zero - e)

    def _ref(in0, in1, c0, c1, c2):
        P = in0.shape[0]
        a = np.asarray(in0, dtype=np.float32).reshape(P, -1)
        x = np.asarray(in1, dtype=np.float32).reshape(P, -1)
        bb = np.abs(a * x - c0).astype(np.float32)
        return bb, bb.sum(axis=-1, keepdims=True)

    return _register_op("MUL_ABSDIFF_SUM_ANT", _body, _ref)


def build_nc():
    mad_op = _register_mul_absdiff_op()
    nc = bacc.Bacc(
        "TRN2", target_bir_lowering=False, debug=False, enable_asserts=False
    )
    real = nc.dram_tensor("real", [BS, FF], F32, kind="ExternalInput").ap()
    fake = nc.dram_tensor("fake", [BS, FF], F32, kind="ExternalInput").ap()
    fw = nc.dram_tensor("fw", [BS, PP], F32, kind="ExternalInput").ap()
    out = nc.dram_tensor("out", [128, NCOL], F32, kind="ExternalOutput").ap()

    with tile.TileContext(nc) as tc:
        with (
            tc.tile_pool(name="small", bufs=1) as sp,
            tc.tile_pool(name="xr", bufs=6) as xr_pool,
            tc.tile_pool(name="xfc", bufs=4) as xfc_pool,
            tc.tile_pool(name="t", bufs=2) as t_pool,
        ):
            O = sp.tile([128, NCOL], F32)

            # --- fw via the Sync HWDGE ring, ahead of the fake chunks
            fwt = sp.tile([128, PP], F32)

            # fake-chunk loads on the Sync ring: prefetch a few, then issue
            # the rest one per loop iteration (just-in-time, baseline-style)
            xfc_tiles = [None] * NCH
            PRE_F = 6

            def issue_fake(c):
                lo, sz = CHUNKS[c]
                xfc = xfc_pool.tile([128, 2 * RF], F32, tag="xfc")
                nc.sync.dma_start(xfc[:, 0:sz], fake[:, lo : lo + sz])
                xfc_tiles[c] = xfc

            for c in range(PRE_F):
                issue_fake(c)

            # first few real-chunk loads on the Scalar ring
            xr_tiles = [None] * NCH
            PRE_R = 4

            def issue_real(c):
                lo, sz = CHUNKS[c]
                xr = xr_pool.tile([128, RF], F32, tag="xr")
                nc.scalar.dma_start(xr[:, 0:sz], real[:, lo : lo + sz])
                xr_tiles[c] = xr

            for c in range(PRE_R):
                issue_real(c)

            # s prep: s = +/-1 from fw > 0; b45 = -0.45*s (per-row act bias)
            g_t = sp.tile([128, PP], F32)
            nc.vector.tensor_scalar(g_t[:], fwt[:], 0.0, None, ALU.is_gt)
            s_t = sp.tile([128, PP], F32)
            nc.vector.tensor_scalar(s_t[:], g_t[:], 2.0, 1.0, ALU.mult, ALU.subtract)
            b45 = sp.tile([128, PP], F32)
            nc.vector.tensor_scalar(b45[:], s_t[:], -0.45, None, ALU.mult)
            b45 = sp.tile([128, PP], F32)
            nc.vector.tensor_scalar(b45[:], s_t[:], -0.45, None, ALU.mult)

            diag = sp.tile([128, PP], F32)
            sd = sp.tile([128, 2 * RF], F32)   # custom-op out scratch
            sa = sp.tile([128, RF], F32)    # ScalarE activation out scratch
            sd = sp.tile([128, RF], F32)    # VectorE custom-op out scratch
            b08 = sp.tile([128, 1], F32)    # activation bias (-0.8)
            nc.vector.memset(b08[:], -0.8)
            ones = sp.tile([128, 1], F32)   # broadcast 1.0 for the D8 reduce
            nc.vector.memset(ones[:], 1.0)

            def fake_pass1(c):
                lo, sz = CHUNKS[c]
                rows = sz // PP
                r0 = lo // PP
                t = t_pool.tile([128, 2 * RF], F32, tag="t")
                if c in SC_P1:
                    # per-row on ScalarE: t_i = s_i*x_i - 0.45*s_i
                    for i in range(rows):
                        nc.scalar.activation(
                            t[:, i * PP : (i + 1) * PP],
                            xfc_tiles[c][:, i * PP : (i + 1) * PP],
                            AF.Identity,
                            bias=b45[:, r0 + i : r0 + i + 1],
                            scale=s_t[:, r0 + i : r0 + i + 1],
                        )
                else:
                    x3 = xfc_tiles[c][:, 0:sz].rearrange(
                        "p (i j) -> p i j", j=PP
                    )
                    t3 = t[:, 0:sz].rearrange("p (i j) -> p i j", j=PP)
                    sjb = s_t[:].rearrange("p j -> p () j").to_broadcast(
                        [128, rows, PP]
                    )
                    nc.vector.scalar_tensor_tensor(
                        t3, x3, 0.45, sjb, ALU.subtract, ALU.mult
                    )
                return t

            def fake_pass2(c, t):
                lo, sz = CHUNKS[c]
                rows = sz // PP
                r0 = lo // PP
                t3 = t[:, 0:sz].rearrange("p (i j) -> p i j", j=PP)
                if c in SC_P1:
                    # pass 1 folded s_i; fold s_j here
                    sb = s_t[:].rearrange("p j -> p () j").to_broadcast(
                        [128, rows, PP]
                    )
                else:
                    sb = (
                        s_t[:, r0 : r0 + rows]
                        .rearrange("p i -> p i ()")
                        .to_broadcast([128, rows, PP])
                    )
                nc.vector._custom_dve(
                    mad_op,
                    out=sd[:, 0:sz].rearrange("p (i j) -> p i j", j=PP),
                    in0=t3,
                    in1=sb,
                    s0=0.35,
                    accum_out=O[:, COL_FAKE + c : COL_FAKE + c + 1],
                )

            def diag_copy(c):
                lo, sz = CHUNKS[c]
                i0 = -(-lo // 197)
                i1 = -(-(lo + sz) // 197)
                off = 197 * i0 - lo
                cnt = i1 - i0
                nc.vector.tensor_copy(
                    diag[:, i0:i1],
                    xr_tiles[c][:, off : off + 197 * (cnt - 1) + 1 : 197],
                )

            def real_act(c):
                lo, sz = CHUNKS[c]
                nc.scalar.activation(
                    sa[:, 0:sz], xr_tiles[c][:, 0:sz], AF.Abs, bias=b08[:],
                    accum_out=O[:, COL_REAL + c : COL_REAL + c + 1],
                )

            t_tiles = [None] * NCH
            for c in range(NCH):
                t_tiles[c] = fake_pass1(c)
                real_act(c)
                if c + PRE_R < NCH:
                    issue_real(c + PRE_R)
                if c + PRE_F < NCH:
                    issue_fake(c + PRE_F)
                if c >= 1:
                    diag_copy(c - 1)
                    fake_pass2(c - 1, t_tiles[c - 1])
                    t_tiles[c - 1] = None
            diag_copy(NCH - 1)
            fake_pass2(NCH - 1, t_tiles[NCH - 1])

            # diagonal corrections on ScalarE (real diag target is 1.0):
            # ship sum(diag) and sum|diag - 0.8|; host applies 196 - sum(d).
            t1 = sp.tile([128, PP], F32)
            nc.scalar.activation(
                t1[:], diag[:], AF.Abs, bias=b00[:],
                accum_out=O[:, COL_SD : COL_SD + 1],
            )
            t2 = sp.tile([128, PP], F32)
            nc.scalar.activation(
                t2[:], diag[:], AF.Abs, bias=b08[:],
                accum_out=O[:, COL_D8 : COL_D8 + 1],
            )

            nc.sync.dma_start(out[:, :], O[:])

    nc.compile()
    return nc


def _get_nc():
    if "nc" not in _NC_CACHE:
        _NC_CACHE["nc"] = build_nc()
    return _NC_CACHE["nc"]


def make_in_maps(correlation_map_real, correlation_map_fake, fake_weight):
    r = np.ascontiguousarray(correlation_map_real, dtype=np.float32).reshape(B, FF)
    f = np.ascontiguousarray(correlation_map_fake, dtype=np.float32).reshape(B, FF)
    w = np.ascontiguousarray(fake_weight, dtype=np.float32).reshape(B, PP)
    return [
        {
            "real": r[k * BS : (k + 1) * BS],
            "fake": f[k * BS : (k + 1) * BS],
            "fw": w[k * BS : (k + 1) * BS],
        }
        for k in range(NCORES)
    ]


def reduce_outputs(results):
    total = 0.0
    for k in range(NCORES):
        Ov = results[k]["out"].astype(np.float64)
        total += (
            Ov[:, COL_REAL : COL_REAL + NCH].sum()
            + Ov[:, COL_FAKE : COL_FAKE + NFCH].sum()
            + (BS * PP - Ov[:, COL_SD].sum())   # sum(1 - d) over the diag
            - Ov[:, COL_D8].sum()
        )
    return np.float32(total / DENOM)


def run(inputs, trace=False, **kwargs):
    nc = _get_nc()
    in_maps = make_in_maps(**inputs)
    res = bass_utils.run_bass_kernel_spmd(
        nc, in_maps, list(range(NCORES)), trace=trace, **kwargs
    )
    return reduce_outputs(res.results), res


def kernel(correlation_map_real, correlation_map_fake, fake_weight):
    loss, _ = run(
        dict(
            correlation_map_real=correlation_map_real,
            correlation_map_fake=correlation_map_fake,
            fake_weight=fake_weight,
        )
    )
    return loss


# revision 29
# speedup vs baseline: 1.1464x; 1.0147x over previous
"""Trainium2 Bass kernel for nn_Attention_Correlation_weight_reshape_loss.

loss = [ sum|real - C_real| + sum|fake - C_fake| ] / (B*(PP^2-PP))

Key identity: C_IN == C_OUT == 0.8, so with s[b,i] = +1 if fake_weight[b,i] > 0
else -1 the fake target is rank-1:
    C_fake[b,i,j] = 0.45 + 0.35 * s[b,i] * s[b,j]
and since s*s = +/-1:
    |fake - C_fake| = | (fake - 0.45)*s_i*s_j - 0.35 |
C_real = 0.8 everywhere except the diagonal (1.0); since x < 1, |x_d - 1| =
1 - x_d, so the diagonal correction needs only sum(diag) and sum|diag - 0.8|.

Per-core plan (data-parallel over batch, 8 cores x 128 batches), both maps in
the flat [batch=partition, 38416] layout -> every DMA descriptor is a >=2352 B
contiguous run. Two HWDGE rings stream in parallel (measured ~425 GB/s
aggregate): fake chunks on the Sync ring, real chunks on the Scalar ring, with
each ring's issue slots emitted so no data-waiting compute ever sits ahead of
a DMA issue on the same engine queue.

VectorE is the scarce engine (both fake passes are DVE work; 1 elem/cycle in
every dtype -- 16-bit gets no DVE speedup, and GpSimd elementwise throttles
the DVE via the shared SBUF ports, both measured). Balance:
  ScalarE:  real chunks Abs(x-0.8) activation w/ accumulate, and for a few
            fake chunks pass 1 done per-row: for one row s_i is a
            per-partition scalar, so Identity activation with
            scale=s[:,i], bias=-0.45*s[:,i] gives t = (x-0.45)*s_i.
  VectorE:  fake pass 1 via fused STT for the remaining chunks, fake pass 2
            |t * s - 0.35| custom DVE op w/ accumulate for all chunks
            (the sign not folded in pass 1 rides pass 2's broadcast),
            per-chunk diagonal gather + 2 tiny end reductions.
No compute op writes a DMA-landing tile in place (separate scratch outputs),
so buffer recycling never crosses engines on the critical path.
Host sums the [128, 36] partial tensor from each core and divides by denom.
"""

from operator import add as _op_add

import numpy as np

import concourse.bacc as bacc
import concourse.bass as bass
import concourse.mybir as mybir
import concourse.tile as tile
from concourse import bass_utils
from concourse import dve_ops as _dops
from concourse.dve_spec import Spec, Src0, Src1, Zero, maxx, lower
from concourse.dve_spec import _has_src1
from concourse import dve_spec as _dspec
from concourse.dve_uop import DveOpSpec


def _ensure_axon_ntff_shim():
    """Some agent images lack antenv.axon_hooks; run_bass_kernel_spmd
    (trace=True under axon) hard-imports it. Install a minimal shim wired
    to the axon .so so tracing works instead of crashing."""
    import sys
    import types

    try:
        import antenv.axon_hooks  # noqa: F401
        return
    except ImportError:
        pass
    try:
        import antenv
    except ImportError:
        return
    mod = types.ModuleType("antenv.axon_hooks")
    _hook = [None]
    mod.set_axon_ntff_profile_hook = lambda h: _hook.__setitem__(0, h)
    mod.get_axon_ntff_profile_hook = lambda: _hook[0]
    sys.modules["antenv.axon_hooks"] = mod
    antenv.axon_hooks = mod
    try:
        from trn_agent_boot.trn_boot import _ntff_profile_via_ctypes

        mod.set_axon_ntff_profile_hook(
            _ntff_profile_via_ctypes("/opt/axon/libaxon_pjrt.so")
        )
    except Exception:
        pass


_ensure_axon_ntff_shim()

F32 = mybir.dt.float32
AF = mybir.ActivationFunctionType
ALU = mybir.AluOpType

B, PP = 1024, 196
NCORES = 8
BS = B // NCORES            # 128 batches per core
FF = PP * PP                # 38416
RF = 2744                   # full chunk: 14 rows of 196

# chunk schedule: two 7-row lead-in chunks (compute starts sooner), 12 full
# 14-row chunks, then 7/4/3-row tail pieces (fine drain). All boundaries are
# multiples of 196 so the fake-map row structure stays intact.
CHUNKS = (
    [(0, 7 * PP), (7 * PP, 7 * PP)]
    + [(RF + c * RF, RF) for c in range(12)]
    + [(13 * RF, 7 * PP), (13 * RF + 7 * PP, 4 * PP), (13 * RF + 11 * PP, 3 * PP)]
)
NCH = len(CHUNKS)           # 17

# fake chunks whose pass 1 runs per-row on ScalarE (full chunks, spread out)
SC_P1 = frozenset({3, 9})

# output partials layout: [128, NCOL]
COL_REAL = 0                # NCH cols: per-chunk sum|x-0.8| (incl diag)
COL_FAKE = NCH              # NCH cols: per-chunk fake sums
COL_SD = 2 * NCH            # sum(diag)        (diag target 1.0: |d-1| = 1-d)
COL_D8 = COL_SD + 1         # sum|diag-0.8|
NCOL = COL_D8 + 1           # 36

DENOM = float(B) * (FF - PP)

_NC_CACHE = {}


def _register_op(name, body_fn, ref_fn):
    for op in _dops.OPS:
        if op.name == name:
            return op
    spec = Spec(body=body_fn(), accum=_op_add, accum_init=Zero, reference=ref_fn)
    row = max(_dops._SUB_OPCODE_FOR_NAME.values()) + 1
    assert row < 0x20
    _dops._SUB_OPCODE_FOR_NAME[name] = row
    shas = {}
    for ver in ("v3", "v4"):
        s = DveOpSpec(
            name=name, opcode=row, uops=lower(spec, ver=ver),
            rd1_en=_has_src1(spec),
        )
        shas[ver] = s.sha(ver)
    op = _dops.DveOp(name, spec, subdim=False, uops_sha=shas)
    _dops.OPS.append(op)
    _dops.CUSTOM_DVE_SPECS[name] = spec
    return op


def _register_mul_absdiff_op():
    """out = |in0*in1 - s0|, accum_out = row-sum(out)."""

    def _body():
        e = (Src0 * Src1) - _dspec.C0
        return maxx(e, Zero - e)

    def _ref(in0, in1, c0, c1, c2):
        P = in0.shape[0]
        a = np.asarray(in0, dtype=np.float32).reshape(P, -1)
        x = np.asarray(in1, dtype=np.float32).reshape(P, -1)
        bb = np.abs(a * x - c0).astype(np.float32)
        return bb, bb.sum(axis=-1, keepdims=True)

    return _register_op("MUL_ABSDIFF_SUM_ANT", _body, _ref)


def build_nc():
    mad_op = _register_mul_absdiff_op()
    nc = bacc.Bacc(
        "TRN2", target_bir_lowering=False, debug=False, enable_asserts=False
    )
    real = nc.dram_tensor("real", [BS, FF], F32, kind="ExternalInput").ap()
    fake = nc.dram_tensor("fake", [BS, FF], F32, kind="ExternalInput").ap()
    fw = nc.dram_tensor("fw", [BS, PP], F32, kind="ExternalInput").ap()
    out = nc.dram_tensor("out", [128, NCOL], F32, kind="ExternalOutput").ap()

    with tile.TileContext(nc) as tc:
        with (
            tc.tile_pool(name="small", bufs=1) as sp,
            tc.tile_pool(name="xr", bufs=6) as xr_pool,
            tc.tile_pool(name="xfc", bufs=4) as xfc_pool,
            tc.tile_pool(name="t", bufs=2) as t_pool,
        ):
            O = sp.tile([128, NCOL], F32)

            # --- fw via the Sync HWDGE ring, ahead of the fake chunks
            fwt = sp.tile([128, PP], F32)

            # fake-chunk loads on the Sync ring: prefetch a few, then issue
            # the rest one per loop iteration (just-in-time, baseline-style)
            xfc_tiles = [None] * NCH
            PRE_F = 6

            def issue_fake(c):
                lo, sz = CHUNKS[c]
                xfc = xfc_pool.tile([128, 2 * RF], F32, tag="xfc")
                nc.sync.dma_start(xfc[:, 0:sz], fake[:, lo : lo + sz])
                xfc_tiles[c] = xfc

            for c in range(PRE_F):
                issue_fake(c)

            # first few real-chunk loads on the Scalar ring
            xr_tiles = [None] * NCH
            PRE_R = 4

            def issue_real(c):
                lo, sz = CHUNKS[c]
                xr = xr_pool.tile([128, RF], F32, tag="xr")
                eng = nc.scalar if c % 2 == 0 else nc.sync
                eng.dma_start(xr[:, 0:sz], real[:, lo : lo + sz])
                xr_tiles[c] = xr

            for c in range(PRE_R):
                issue_real(c)

            # s prep: s = +/-1 from fw > 0; b45 = -0.45*s (per-row act bias)
            g_t = sp.tile([128, PP], F32)
            nc.vector.tensor_scalar(g_t[:], fwt[:], 0.0, None, ALU.is_gt)
            s_t = sp.tile([128, PP], F32)
            nc.vector.tensor_scalar(s_t[:], g_t[:], 2.0, 1.0, ALU.mult, ALU.subtract)
            b45 = sp.tile([128, PP], F32)
            nc.vector.tensor_scalar(b45[:], s_t[:], -0.45, None, ALU.mult)
            b45 = sp.tile([128, PP], F32)
            nc.vector.tensor_scalar(b45[:], s_t[:], -0.45, None, ALU.mult)

            diag = sp.tile([128, PP], F32)
            sd = sp.tile([128, 2 * RF], F32)   # custom-op out scratch
            sa = sp.tile([128, RF], F32)    # ScalarE activation out scratch
            sd = sp.tile([128, RF], F32)    # VectorE custom-op out scratch
            b08 = sp.tile([128, 1], F32)    # activation bias (-0.8)
            nc.vector.memset(b08[:], -0.8)
            ones = sp.tile([128, 1], F32)   # broadcast 1.0 for the D8 reduce
            nc.vector.memset(ones[:], 1.0)

            def fake_pass1(c):
                lo, sz = CHUNKS[c]
                rows = sz // PP
                r0 = lo // PP
                t = t_pool.tile([128, 2 * RF], F32, tag="t")
                if c in SC_P1:
                    # per-row on ScalarE: t_i = s_i*x_i - 0.45*s_i
                    for i in range(rows):
                        nc.scalar.activation(
                            t[:, i * PP : (i + 1) * PP],
                            xfc_tiles[c][:, i * PP : (i + 1) * PP],
                            AF.Identity,
                            bias=b45[:, r0 + i : r0 + i + 1],
                            scale=s_t[:, r0 + i : r0 + i + 1],
                        )
                else:
                    x3 = xfc_tiles[c][:, 0:sz].rearrange(
                        "p (i j) -> p i j", j=PP
                    )
                    t3 = t[:, 0:sz].rearrange("p (i j) -> p i j", j=PP)
                    sjb = s_t[:].rearrange("p j -> p () j").to_broadcast(
                        [128, rows, PP]
                    )
                    nc.vector.scalar_tensor_tensor(
                        t3, x3, 0.45, sjb, ALU.subtract, ALU.mult
                    )
                return t

            def fake_pass2(c, t):
                lo, sz = CHUNKS[c]
                rows = sz // PP
                r0 = lo // PP
                t3 = t[:, 0:sz].rearrange("p (i j) -> p i j", j=PP)
                if c in SC_P1:
                    # pass 1 folded s_i; fold s_j here
                    sb = s_t[:].rearrange("p j -> p () j").to_broadcast(
                        [128, rows, PP]
                    )
                else:
                    sb = (
                        s_t[:, r0 : r0 + rows]
                        .rearrange("p i -> p i ()")
                        .to_broadcast([128, rows, PP])
                    )
                nc.vector._custom_dve(
                    mad_op,
                    out=sd[:, 0:sz].rearrange("p (i j) -> p i j", j=PP),
                    in0=t3,
                    in1=sb,
                    s0=0.35,
                    accum_out=O[:, COL_FAKE + c : COL_FAKE + c + 1],
                )

            def diag_copy(c):
                lo, sz = CHUNKS[c]
                i0 = -(-lo // 197)
                i1 = -(-(lo + sz) // 197)
                off = 197 * i0 - lo
                cnt = i1 - i0
                nc.vector.tensor_copy(
                    diag[:, i0:i1],
                    xr_tiles[c][:, off : off + 197 * (cnt - 1) + 1 : 197],
                )

            def real_act(c):
                lo, sz = CHUNKS[c]
                nc.scalar.activation(
                    sa[:, 0:sz], xr_tiles[c][:, 0:sz], AF.Abs, bias=b08[:],
                    accum_out=O[:, COL_REAL + c : COL_REAL + c + 1],
                )

            t_tiles = [None] * NCH
            for c in range(NCH):
                t_tiles[c] = fake_pass1(c)
                real_act(c)
                if c + PRE_R < NCH:
                    issue_real(c + PRE_R)
                if c + PRE_F < NCH:
                    issue_fake(c + PRE_F)
                if c >= 1:
                    diag_copy(c - 1)
                    fake_pass2(c - 1, t_tiles[c - 1])
                    t_tiles[c - 1] = None
            diag_copy(NCH - 1)
            fake_pass2(NCH - 1, t_tiles[NCH - 1])

            # diagonal corrections on ScalarE (real diag target is 1.0):
            # ship sum(diag) and sum|diag - 0.8|; host applies 196 - sum(d).
            t1 = sp.tile([128, PP], F32)
            nc.scalar.activation(
                t1[:], diag[:], AF.Abs, bias=b00[:],
                accum_out=O[:, COL_SD : COL_SD + 1],
            )
            t2 = sp.tile([128, PP], F32)
            nc.scalar.activation(
                t2[:], diag[:], AF.Abs, bias=b08[:],
                accum_out=O[:, COL_D8 : COL_D8 + 1],
            )

            nc.sync.dma_start(out[:, :], O[:])

    nc.compile()
    return nc


def _get_nc():
    if "nc" not in _NC_CACHE:
        _NC_CACHE["nc"] = build_nc()
    return _NC_CACHE["nc"]


def make_in_maps(correlation_map_real, correlation_map_fake, fake_weight):
    r = np.ascontiguousarray(correlation_map_real, dtype=np.float32).reshape(B, FF)
    f = np.ascontiguousarray(correlation_map_fake, dtype=np.float32).reshape(B, FF)
    w = np.ascontiguousarray(fake_weight, dtype=np.float32).reshape(B, PP)
    return [
        {
            "real": r[k * BS : (k + 1) * BS],
            "fake": f[k * BS : (k + 1) * BS],
            "fw": w[k * BS : (k + 1) * BS],
        }
        for k in range(NCORES)
    ]


def reduce_outputs(results):
    total = 0.0
    for k in range(NCORES):
        Ov = results[k]["out"].astype(np.float64)
        total += (
            Ov[:, COL_REAL : COL_REAL + NCH].sum()
            + Ov[:, COL_FAKE : COL_FAKE + NFCH].sum()
            + (BS * PP - Ov[:, COL_SD].sum())   # sum(1 - d) over the diag
            - Ov[:, COL_D8].sum()
        )
    return np.float32(total / DENOM)


def run(inputs, trace=False, **kwargs):
    nc = _get_nc()
    in_maps = make_in_maps(**inputs)
    res = bass_utils.run_bass_kernel_spmd(
        nc, in_maps, list(range(NCORES)), trace=trace, **kwargs
    )
    return reduce_outputs(res.results), res


def kernel(correlation_map_real, correlation_map_fake, fake_weight):
    loss, _ = run(
        dict(
            correlation_map_real=correlation_map_real,
            correlation_map_fake=correlation_map_fake,
            fake_weight=fake_weight,
        )
    )
    return loss
